# revision 6
# baseline (speedup 1.0000x reference)
"""Two-layer GAT (DGL GATConv style) on 8 TRN2 NeuronCores via Bass/Tile.

Strategy (node partitioning per the classic graph-partition recipe):
  - Destination nodes are partitioned contiguously across the 8 cores
    (node n -> core n // NPAD). Each core owns the edge-softmax and
    aggregation for edges whose dst it owns.
  - Host pre-sorts edges by dst and groups them into 128-dst "blocks";
    each block's edges are padded to a multiple of 128 (chunks).
  - Per chunk of 128 edges the kernel gathers [feat|el] rows by src and
    er by dst with indirect DMA, builds a 128x128 one-hot(dst) matrix on
    the vector engine, and uses the tensor engine to segment-sum
    exp(leakyrelu(el+er)) - weighted features into PSUM (numerator and
    softmax denominator in one accumulating matmul).
  - Weights are replicated; the projection (X @ W1, plus the small
    attention reductions folded into extra matmul columns) is computed
    redundantly on every core. Between layers, the per-core x1 shards
    are exchanged with a single AllGather (transposed layout so layer-2
    projection needs no on-device transpose).
"""

import sys

sys.path.insert(0, "/opt/trn_rl_repo")

import numpy as np

import concourse.bass as bass
import concourse.mybir as mybir
from concourse import bacc, tile
from concourse.masks import make_identity

F32 = mybir.dt.float32
I32 = mybir.dt.int32
AF = mybir.ActivationFunctionType
OP = mybir.AluOpType

IN_DIM, HID, HEADS, OUT_DIM = 128, 32, 4, 16
NEG_SLOPE = 0.2
NCORES = 8
P = 128
EPS = 1e-30

T1W = IN_DIM + 2 * HEADS  # 136: table1 row = [feat(128) | el(4) | er(4)]
G1W = IN_DIM + HEADS      # 132: gathered/rhs chunk = [feat(128) | x(4)]
T2W = OUT_DIM + 2         # 18:  table2 row = [feat2(16) | el2(1) | er2(1)]
G2W = OUT_DIM + 1         # 17:  gathered/rhs chunk = [feat2(16) | x(1)]


def build_program(npad: int, nch: int):
    """Build the single SPMD Bass program (same NEFF on all 8 cores)."""
    nblk = npad // P
    ntot = NCORES * npad
    ntiles = ntot // P

    nc = bacc.Bacc(num_devices=NCORES)
    featT = nc.declare_dram_parameter("featT", [P, ntot], F32, isOutput=False)
    r1 = nc.declare_dram_parameter("R1", [P, T1W], F32, isOutput=False)
    r2 = nc.declare_dram_parameter("R2", [HID, T2W], F32, isOutput=False)
    b1r = nc.declare_dram_parameter("b1rep4", [P, IN_DIM], F32, isOutput=False)
    b2r = nc.declare_dram_parameter("b2rep", [P, OUT_DIM], F32, isOutput=False)
    srcidx = nc.declare_dram_parameter("srcidx", [nblk, P, nch], I32, isOutput=False)
    dstidx = nc.declare_dram_parameter("dstidx", [nblk, P, nch], I32, isOutput=False)
    dstloc = nc.declare_dram_parameter("dstloc", [nblk, P, nch], F32, isOutput=False)
    out = nc.declare_dram_parameter("out", [npad, OUT_DIM], F32, isOutput=True)

    with tile.TileContext(nc) as tc:
        with (
            tc.tile_pool(name="dram", bufs=1, space="DRAM") as dpool,
            tc.tile_pool(name="const", bufs=1) as cpool,
        ):
            table1 = dpool.tile([ntot, T1W], F32)
            table2 = dpool.tile([ntot, T2W], F32)
            x1t_own = dpool.tile([HID, npad], F32)
            x1t_full = dpool.tile([NCORES * HID, npad], F32, addr_space="Shared")

            r1_sb = cpool.tile([P, T1W], F32)
            nc.sync.dma_start(out=r1_sb[:], in_=r1[:, :])
            r2_sb = cpool.tile([HID, T2W], F32)
            nc.sync.dma_start(out=r2_sb[:], in_=r2[:, :])
            b1_sb = cpool.tile([P, IN_DIM], F32)
            nc.sync.dma_start(out=b1_sb[:], in_=b1r[:, :])
            b2_sb = cpool.tile([P, OUT_DIM], F32)
            nc.sync.dma_start(out=b2_sb[:], in_=b2r[:, :])
            iota_i = cpool.tile([P, P], I32)
            nc.gpsimd.iota(out=iota_i[:], pattern=[[1, P]], base=0, channel_multiplier=0)
            iota_f = cpool.tile([P, P], F32)
            nc.vector.tensor_copy(out=iota_f[:], in_=iota_i[:])
            ident = cpool.tile([P, P], F32)
            make_identity(nc, ident[:])

            # ---- phase A: table1 = [X@W1 | el | er] for every node ----
            with (
                tc.tile_pool(name="pa", bufs=4) as pa,
                tc.tile_pool(name="pap", bufs=4, space="PSUM") as pap,
            ):
                for t in range(ntiles):
                    xt = pa.tile([P, P], F32, tag="xt")
                    nc.sync.dma_start(out=xt[:], in_=featT[:, t * P:(t + 1) * P])
                    ps = pap.tile([P, T1W], F32, tag="ps")
                    nc.tensor.matmul(out=ps[:], lhsT=xt[:], rhs=r1_sb[:], start=True, stop=True)
                    tb = pa.tile([P, T1W], F32, tag="tb")
                    nc.vector.tensor_copy(out=tb[:], in_=ps[:])
                    nc.sync.dma_start(out=table1[t * P:(t + 1) * P, :], in_=tb[:])

            # ---- phase B: layer-1 edge softmax + aggregation per dst block ----
            with (
                tc.tile_pool(name="pb", bufs=2) as pb,
                tc.tile_pool(name="pbs", bufs=3) as pbs,
                tc.tile_pool(name="pbm", bufs=4) as pbm,
                tc.tile_pool(name="pbp", bufs=2, space="PSUM") as pbp,
            ):
                for b in range(nblk):
                    isx = pbs.tile([P, nch], I32, tag="isx")
                    nc.sync.dma_start(out=isx[:], in_=srcidx[b, :, :])
                    idx = pbs.tile([P, nch], I32, tag="idx")
                    nc.sync.dma_start(out=idx[:], in_=dstidx[b, :, :])
                    dlc = pbs.tile([P, nch], F32, tag="dlc")
                    nc.sync.dma_start(out=dlc[:], in_=dstloc[b, :, :])

                    # this walrus lowers indirect DMA as one index per
                    # partition, so gather chunk-by-chunk with [P,1] indices
                    g = pb.tile([P, nch * G1W], F32, tag="g")
                    ger = pbs.tile([P, nch * HEADS], F32, tag="ger")
                    for c in range(nch):
                        nc.gpsimd.indirect_dma_start(
                            out=g[:, c * G1W:(c + 1) * G1W], out_offset=None,
                            in_=table1[:, :],
                            in_offset=bass.IndirectOffsetOnAxis(ap=isx[:, c:c + 1], axis=0),
                        )
                        nc.gpsimd.indirect_dma_start(
                            out=ger[:, c * HEADS:(c + 1) * HEADS], out_offset=None,
                            in_=table1[:, :],
                            in_offset=bass.IndirectOffsetOnAxis(ap=idx[:, c:c + 1], axis=0),
                            element_offset=IN_DIM + HEADS,
                        )
                    gv = g[:].rearrange("p (c w) -> p c w", w=G1W)
                    et = pbs.tile([P, nch * HEADS], F32, tag="et")
                    nc.vector.tensor_tensor(
                        out=et[:].rearrange("p (c w) -> p c w", w=HEADS),
                        in0=gv[:, :, IN_DIM:G1W],
                        in1=ger[:].rearrange("p (c w) -> p c w", w=HEADS),
                        op=OP.add,
                    )
                    xt_ = pbs.tile([P, nch * HEADS], F32, tag="xt_")
                    nc.vector.tensor_scalar(out=xt_[:], in0=et[:], scalar1=NEG_SLOPE,
                                            scalar2=None, op0=OP.mult)
                    nc.vector.tensor_tensor(out=xt_[:], in0=xt_[:], in1=et[:], op=OP.max)
                    xr = pb.tile([P, nch * G1W], F32, tag="xr")
                    xrv = xr[:].rearrange("p (c w) -> p c w", w=G1W)
                    nc.scalar.activation(
                        out=xrv[:, :, IN_DIM:G1W],
                        in_=xt_[:].rearrange("p (c w) -> p c w", w=HEADS),
                        func=AF.Exp,
                    )
                    up = pbp.tile([P, G1W], F32, tag="up")
                    for c in range(nch):
                        m = pbm.tile([P, P], F32, tag="m")
                        nc.vector.tensor_scalar(out=m[:], in0=iota_f[:],
                                                scalar1=dlc[:, c:c + 1],
                                                scalar2=None, op0=OP.is_equal)
                        base = c * G1W
                        for h in range(HEADS):
                            nc.vector.tensor_scalar(
                                out=xr[:, base + h * HID: base + (h + 1) * HID],
                                in0=g[:, base + h * HID: base + (h + 1) * HID],
                                scalar1=xr[:, base + IN_DIM + h: base + IN_DIM + h + 1],
                                scalar2=None, op0=OP.mult)
                        nc.tensor.matmul(out=up[:], lhsT=m[:], rhs=xr[:, base: base + G1W],
                                         start=(c == 0), stop=(c == nch - 1))
                    u = pbs.tile([P, G1W], F32, tag="u")
                    nc.vector.tensor_copy(out=u[:], in_=up[:])
                    rs = pbs.tile([P, HEADS], F32, tag="rs")
                    nc.vector.tensor_scalar(out=rs[:], in0=u[:, IN_DIM:G1W], scalar1=EPS,
                                            scalar2=None, op0=OP.add)
                    nc.vector.reciprocal(out=rs[:], in_=rs[:])
                    nc.vector.tensor_scalar(out=rs[:], in0=rs[:], scalar1=1.0 / HEADS,
                                            scalar2=None, op0=OP.mult)
                    v = pbs.tile([P, IN_DIM], F32, tag="v")
                    for h in range(HEADS):
                        nc.vector.tensor_scalar(out=v[:, h * HID:(h + 1) * HID],
                                                in0=u[:, h * HID:(h + 1) * HID],
                                                scalar1=rs[:, h:h + 1],
                                                scalar2=None, op0=OP.mult)
                    nc.vector.tensor_tensor(out=v[:], in0=v[:], in1=b1_sb[:], op=OP.add)
                    nc.vector.tensor_scalar(out=v[:], in0=v[:], scalar1=0.0,
                                            scalar2=None, op0=OP.max)
                    x1 = pbs.tile([P, HID], F32, tag="x1")
                    nc.vector.tensor_tensor(out=x1[:], in0=v[:, 0:HID],
                                            in1=v[:, HID:2 * HID], op=OP.add)
                    nc.vector.tensor_tensor(out=x1[:], in0=x1[:],
                                            in1=v[:, 2 * HID:3 * HID], op=OP.add)
                    nc.vector.tensor_tensor(out=x1[:], in0=x1[:],
                                            in1=v[:, 3 * HID:4 * HID], op=OP.add)
                    tp = pbp.tile([HID, P], F32, tag="tp")
                    nc.tensor.transpose(out=tp[:], in_=x1[:], identity=ident[:])
                    x1t = pbs.tile([HID, P], F32, tag="x1t")
                    nc.vector.tensor_copy(out=x1t[:], in_=tp[:])
                    nc.sync.dma_start(out=x1t_own[:, b * P:(b + 1) * P], in_=x1t[:])

            # ---- phase C: exchange x1 (transposed) across cores ----
            nc.gpsimd.collective_compute(
                "AllGather", OP.bypass,
                replica_groups=[list(range(NCORES))],
                ins=[x1t_own[:, :]],
                outs=[x1t_full[:, :]],
            )

            # ---- phase D: table2 = [x1@W2 | el2 | er2] for every node ----
            with (
                tc.tile_pool(name="pd", bufs=4) as pd,
                tc.tile_pool(name="pdp", bufs=4, space="PSUM") as pdp,
            ):
                for t in range(ntiles):
                    corei = (t * P) // npad
                    off = (t * P) % npad
                    xt2 = pd.tile([HID, P], F32, tag="xt2")
                    nc.sync.dma_start(
                        out=xt2[:],
                        in_=x1t_full[corei * HID:(corei + 1) * HID, off:off + P])
                    ps2 = pdp.tile([P, T2W], F32, tag="ps2")
                    nc.tensor.matmul(out=ps2[:], lhsT=xt2[:], rhs=r2_sb[:],
                                     start=True, stop=True)
                    tb2 = pd.tile([P, T2W], F32, tag="tb2")
                    nc.vector.tensor_copy(out=tb2[:], in_=ps2[:])
                    nc.sync.dma_start(out=table2[t * P:(t + 1) * P, :], in_=tb2[:])

            # ---- phase E: layer-2 edge softmax/aggregation + log_softmax ----
            with (
                tc.tile_pool(name="pe", bufs=2) as pe,
                tc.tile_pool(name="pes", bufs=3) as pes,
                tc.tile_pool(name="pem", bufs=4) as pem,
                tc.tile_pool(name="pep", bufs=2, space="PSUM") as pep,
            ):
                for b in range(nblk):
                    isx2 = pes.tile([P, nch], I32, tag="isx2")
                    nc.sync.dma_start(out=isx2[:], in_=srcidx[b, :, :])
                    idx2 = pes.tile([P, nch], I32, tag="idx2")
                    nc.sync.dma_start(out=idx2[:], in_=dstidx[b, :, :])
                    dlc2 = pes.tile([P, nch], F32, tag="dlc2")
                    nc.sync.dma_start(out=dlc2[:], in_=dstloc[b, :, :])

                    g2 = pe.tile([P, nch * G2W], F32, tag="g2")
                    g2e = pes.tile([P, nch], F32, tag="g2e")
                    for c in range(nch):
                        nc.gpsimd.indirect_dma_start(
                            out=g2[:, c * G2W:(c + 1) * G2W], out_offset=None,
                            in_=table2[:, :],
                            in_offset=bass.IndirectOffsetOnAxis(ap=isx2[:, c:c + 1], axis=0),
                        )
                        nc.gpsimd.indirect_dma_start(
                            out=g2e[:, c:c + 1], out_offset=None,
                            in_=table2[:, :],
                            in_offset=bass.IndirectOffsetOnAxis(ap=idx2[:, c:c + 1], axis=0),
                            element_offset=OUT_DIM + 1,
                        )
                    g2v = g2[:].rearrange("p (c w) -> p c w", w=G2W)
                    et2 = pes.tile([P, nch], F32, tag="et2")
                    nc.vector.tensor_tensor(
                        out=et2[:].rearrange("p (c w) -> p c w", w=1),
                        in0=g2v[:, :, OUT_DIM:G2W],
                        in1=g2e[:].rearrange("p (c w) -> p c w", w=1),
                        op=OP.add,
                    )
                    xt2_ = pes.tile([P, nch], F32, tag="xt2_")
                    nc.vector.tensor_scalar(out=xt2_[:], in0=et2[:], scalar1=NEG_SLOPE,
                                            scalar2=None, op0=OP.mult)
                    nc.vector.tensor_tensor(out=xt2_[:], in0=xt2_[:], in1=et2[:], op=OP.max)
                    xr2 = pe.tile([P, nch * G2W], F32, tag="xr2")
                    xr2v = xr2[:].rearrange("p (c w) -> p c w", w=G2W)
                    nc.scalar.activation(
                        out=xr2v[:, :, OUT_DIM:G2W],
                        in_=xt2_[:].rearrange("p (c w) -> p c w", w=1),
                        func=AF.Exp,
                    )
                    up2 = pep.tile([P, G2W], F32, tag="up2")
                    for c in range(nch):
                        m2 = pem.tile([P, P], F32, tag="m2")
                        nc.vector.tensor_scalar(out=m2[:], in0=iota_f[:],
                                                scalar1=dlc2[:, c:c + 1],
                                                scalar2=None, op0=OP.is_equal)
                        base = c * G2W
                        nc.vector.tensor_scalar(
                            out=xr2[:, base: base + OUT_DIM],
                            in0=g2[:, base: base + OUT_DIM],
                            scalar1=xr2[:, base + OUT_DIM: base + OUT_DIM + 1],
                            scalar2=None, op0=OP.mult)
                        nc.tensor.matmul(out=up2[:], lhsT=m2[:], rhs=xr2[:, base: base + G2W],
                                         start=(c == 0), stop=(c == nch - 1))
                    u2 = pes.tile([P, G2W], F32, tag="u2")
                    nc.vector.tensor_copy(out=u2[:], in_=up2[:])
                    rs2 = pes.tile([P, 1], F32, tag="rs2")
                    nc.vector.tensor_scalar(out=rs2[:], in0=u2[:, OUT_DIM:G2W], scalar1=EPS,
                                            scalar2=None, op0=OP.add)
                    nc.vector.reciprocal(out=rs2[:], in_=rs2[:])
                    o = pes.tile([P, OUT_DIM], F32, tag="o")
                    nc.vector.tensor_scalar(out=o[:], in0=u2[:, 0:OUT_DIM],
                                            scalar1=rs2[:, 0:1], scalar2=None, op0=OP.mult)
                    nc.vector.tensor_tensor(out=o[:], in0=o[:], in1=b2_sb[:], op=OP.add)
                    mx = pes.tile([P, 1], F32, tag="mx")
                    nc.vector.tensor_reduce(out=mx[:], in_=o[:],
                                            axis=mybir.AxisListType.X, op=OP.max)
                    osh = pes.tile([P, OUT_DIM], F32, tag="osh")
                    nc.vector.tensor_scalar(out=osh[:], in0=o[:], scalar1=mx[:, 0:1],
                                            scalar2=None, op0=OP.subtract)
                    ex = pes.tile([P, OUT_DIM], F32, tag="ex")
                    nc.scalar.activation(out=ex[:], in_=osh[:], func=AF.Exp)
                    se = pes.tile([P, 1], F32, tag="se")
                    nc.vector.tensor_reduce(out=se[:], in_=ex[:],
                                            axis=mybir.AxisListType.X, op=OP.add)
                    lg = pes.tile([P, 1], F32, tag="lg")
                    nc.scalar.activation(out=lg[:], in_=se[:], func=AF.Ln)
                    res = pes.tile([P, OUT_DIM], F32, tag="res")
                    nc.vector.tensor_scalar(out=res[:], in0=osh[:], scalar1=lg[:, 0:1],
                                            scalar2=None, op0=OP.subtract)
                    nc.sync.dma_start(out=out[b * P:(b + 1) * P, :], in_=res[:])

    nc.compile()
    return nc


def prepare_inputs(features, src, dst, W1, al1, ar1, b1, W2, al2, ar2, b2):
    """Host-side sharding/preprocessing. Returns (in_maps, npad, nch, n)."""
    n = features.shape[0]
    src = np.asarray(src, dtype=np.int64)
    dst = np.asarray(dst, dtype=np.int64)
    features = np.asarray(features, dtype=np.float32)

    npad = int(np.ceil(n / (NCORES * P))) * P
    nblk = npad // P
    ntot = NCORES * npad
    pad_node = ntot - 1

    # sort edges by dst; block boundaries in the padded node-id space
    order = np.argsort(dst, kind="stable")
    sdst = dst[order]
    ssrc = src[order]
    core_of = sdst // npad
    blk_of = (sdst - core_of * npad) // P
    lane_of = (sdst - core_of * npad) % P
    gblk = core_of * nblk + blk_of  # globally sorted since dst sorted
    nblocks_tot = NCORES * nblk
    bounds = np.searchsorted(gblk, np.arange(nblocks_tot + 1))
    counts = np.diff(bounds)
    nch = max(1, int(np.ceil(counts.max() / P)))

    srcidx = np.full((NCORES, nblk, P, nch), pad_node, dtype=np.int32)
    dstidx = np.full((NCORES, nblk, P, nch), pad_node, dtype=np.int32)
    dstloc = np.full((NCORES, nblk, P, nch), 200.0, dtype=np.float32)
    # vectorized scatter of the sorted edges into (core, blk, lane, chunk)
    pos_in_blk = np.arange(len(sdst)) - bounds[gblk]
    c_of = pos_in_blk // P
    p_of = pos_in_blk % P
    srcidx[core_of, blk_of, p_of, c_of] = ssrc.astype(np.int32)
    dstidx[core_of, blk_of, p_of, c_of] = sdst.astype(np.int32)
    dstloc[core_of, blk_of, p_of, c_of] = lane_of.astype(np.float32)

    featT = np.zeros((P, ntot), dtype=np.float32)
    featT[:, :n] = features.T

    W1r = W1.reshape(IN_DIM, HEADS, HID)
    A1 = np.einsum("fho,ho->fh", W1r, al1).astype(np.float32)
    B1 = np.einsum("fho,ho->fh", W1r, ar1).astype(np.float32)
    R1 = np.concatenate([W1, A1, B1], axis=1).astype(np.float32)  # [128,136]
    a2 = (W2 @ al2[0]).astype(np.float32)[:, None]
    r2v = (W2 @ ar2[0]).astype(np.float32)[:, None]
    R2 = np.concatenate([W2, a2, r2v], axis=1).astype(np.float32)  # [32,18]
    b1rep4 = np.broadcast_to(b1, (P, IN_DIM)).astype(np.float32) / HEADS
    b2rep = np.broadcast_to(b2, (P, OUT_DIM)).astype(np.float32).copy()
    b1rep4 = np.ascontiguousarray(b1rep4)

    in_maps = []
    for ci in range(NCORES):
        in_maps.append({
            "featT": featT,
            "R1": R1,
            "R2": R2,
            "b1rep4": b1rep4,
            "b2rep": b2rep,
            "srcidx": np.ascontiguousarray(srcidx[ci]),
            "dstidx": np.ascontiguousarray(dstidx[ci]),
            "dstloc": np.ascontiguousarray(dstloc[ci]),
        })
    return in_maps, npad, nch, n


_PROG_CACHE: dict = {}


def run(inputs: dict, trace: bool = False):
    """Compile (cached) + run on the 8 cores. Returns (output, BassKernelResults)."""
    from concourse.bass_utils import run_bass_kernel_spmd

    in_maps, npad, nch, n = prepare_inputs(**inputs)
    key = (npad, nch)
    if key not in _PROG_CACHE:
        _PROG_CACHE[key] = build_program(npad, nch)
    nc = _PROG_CACHE[key]
    res = run_bass_kernel_spmd(nc, in_maps, list(range(NCORES)), trace=trace)
    outs = [res.results[ci]["out"] for ci in range(NCORES)]
    full = np.concatenate(outs, axis=0)[:n]
    return np.ascontiguousarray(full, dtype=np.float32), res


def kernel(**inputs) -> np.ndarray:
    out, _ = run(inputs, trace=False)
    return out
